# revision 18
# baseline (speedup 1.0000x reference)
"""Trainium2 Bass kernel for fused QKV projection + interleaved RoPE.

Problem: X[4, 4096, 2048] @ {Wq, Wk, Wv}[2048, 2048] -> reshape to heads
[B, S, 16, 128], apply interleaved RoPE to Q and K, return (Xq, Xk, Xv).

Sharding: data-parallel over tokens. The 4*4096 = 16384 token rows are
split into 8 contiguous shards of 2048 rows (core c gets batch c//2,
sequence half c%2). Every core holds the full Wq/Wk/Wv and computes all
2048 output features for its rows; RoPE is per-token elementwise so no
communication is needed.

Device kernel (identical SPMD program on all 8 cores):
  - X^T shard stays resident in SBUF as 16 per-row-chunk [128k, 128r]
    stationary tiles. The host pre-tiles X into (rc, p, ko, r) blocks so
    every X DMA is a contiguous [128 x 4KB-line] transfer (256B-run
    layouts measured at half DMA rate).
  - matmul out = lhsT.T @ rhs with lhsT = X^T tile (stationary) and
    rhs = W tile [128k, 1024m] (moving, 512-wide slices), accumulating
    psum[128r, 1024m] fp32 over 16 k-chunks.
  - RoPE in 3 DVE ops on the psum tile: the interleaved pair swap is a
    reversed-stride access pattern, the rotation sign is pre-baked into
    the sin table on the host, and cos/sin broadcast across heads via
    zero-stride APs. V drains are DVE copies too: with no scalar-engine
    ACTIVATE in the program the ACT preamble skips its ~1.3us table
    load, so the first W DMA issues earlier.

Cold start / tail (tuned against NTFF profiles):
  - A few dependency-free warmup matmuls on a memset junk tile run
    during the ~6us engine-init prologue, and more are sprinkled into
    the W-bandwidth-limited first pass, so the PE HAM clock gate
    reaches 8/8 early and never re-throttles (idle >3.4us = re-throttle
    to 1.2GHz).
  - Phase 0 ramps row-chunks in as their X tiles arrive (rc0 at ko0,
    rc1 at ko2, rc2 at ko6, rc3 at ko10, ko-major so W tiles are shared
    the moment they land); skipped ko's are caught up afterwards from
    SBUF-resident W. The in-order PE queue therefore never waits on an
    X tile that is still in flight.
  - Phase order V/Q/Q/K/K/V: the last phase has no RoPE, and its last
    row-chunk drains as two 512-wide bank chunks with stores on
    separate rings, cutting the post-matmul tail.
"""

import numpy as np
import ml_dtypes

import concourse.bass as bass
import concourse.mybir as mybir
import concourse.tile as tile
from concourse import bacc
from concourse.bass import ds, ts
from concourse.bass_utils import run_bass_kernel_spmd

B, S, DIM, H = 4, 4096, 2048, 16
HD = DIM // H           # 128
N_CORES = 8
R = B * S // N_CORES    # 2048 token rows per core
P = 128

BF16 = mybir.dt.bfloat16
F32 = mybir.dt.float32

JUNK_PRE = 6                        # warmup matmuls before the first real one
JOIN = {0: 0, 1: 2, 2: 6, 3: 10}    # phase-0: rc -> first shared-pass ko
SPRINKLE = {0: 2, 1: 2, 2: 1, 3: 1, 4: 1, 5: 1, 6: 1, 10: 1}  # ko -> junk
X_GATED = 6                         # x tiles >= this index load after rc0 drains


def build_nc(K=DIM, M=DIM, rows=R, hd=HD, mm_free=512, m_half=1024, loop_n=1,
             unroll=False):
    """Build the per-core Bass program.

    K: contraction dim, M: output feature dim, rows: token rows per core.
    loop_n > 1 wraps the body in a device-side For_i for benchmarking.
    """
    m_half = min(m_half, M)
    assert K % P == 0 and rows % P == 0 and M % m_half == 0
    assert m_half % mm_free == 0 and m_half % hd == 0
    KO = K // P           # k-chunks
    RC = rows // P        # token row chunks
    HALVES = M // m_half  # weight column phases per tensor
    MJ = m_half // mm_free
    NH = m_half // hd     # heads per column phase
    J = hd // 2           # rotation pairs per head

    nc = bacc.Bacc(None, target_bir_lowering=False)

    # xt is host-pre-tiled: xt[rc*P + p, ko*P + r] = X^T[ko*P + p, rc*P + r]
    xt = nc.dram_tensor("xt", [rows, K], BF16, kind="ExternalInput")
    wq = nc.dram_tensor("wq", [K, M], BF16, kind="ExternalInput")
    wk = nc.dram_tensor("wk", [K, M], BF16, kind="ExternalInput")
    wv = nc.dram_tensor("wv", [K, M], BF16, kind="ExternalInput")
    cosf = nc.dram_tensor("cosf", [rows, hd], F32, kind="ExternalInput")
    ssin = nc.dram_tensor("ssin", [rows, hd], F32, kind="ExternalInput")
    q_out = nc.dram_tensor("q", [rows, M], F32, kind="ExternalOutput")
    k_out = nc.dram_tensor("k", [rows, M], F32, kind="ExternalOutput")
    v_out = nc.dram_tensor("v", [rows, M], F32, kind="ExternalOutput")

    xt_r = xt[:].rearrange("(rc p) (ko r) -> p rc ko r", p=P, r=P)
    cos_r = cosf[:].rearrange("(rc p) d -> p rc d", p=P)
    sin_r = ssin[:].rearrange("(rc p) d -> p rc d", p=P)

    with tile.TileContext(nc) as tc:
        with (
            tc.tile_pool(name="wpool", bufs=2 * (K // P)) as wpool,
            tc.tile_pool(name="xpool", bufs=RC) as xpool,
            tc.tile_pool(name="cpool", bufs=1) as cpool,
            tc.tile_pool(name="opool", bufs=4) as opool,
            tc.tile_pool(name="tpool", bufs=2) as tpool,
            tc.tile_pool(name="jpool", bufs=1) as jpool,
            tc.tile_pool(name="psum", bufs=4, space="PSUM") as pspool,
        ):
            def load_w_tiles(w_r, half):
                # per-ko tiles so the first matmul only waits on 256 KB
                tiles = []
                for ko in range(KO):
                    w_sb = wpool.tile([P, m_half], BF16, tag="w")
                    nc.scalar.dma_start(w_sb[:], w_r[:, ko, ts(half, m_half)])
                    tiles.append(w_sb)
                return tiles

            def lhsT_of(xt_tiles, rc, ko):
                xt = xt_tiles[rc]
                if isinstance(xt, list):  # ko-chunked tile list
                    per = KO // len(xt)
                    return xt[ko // per][:, ko % per]
                return xt[:, ko]

            def emit_rc_ko(psum, w_tiles, xt_tiles, rc, ko, n_done):
                # start/stop are per PSUM bank: each mj slice is its own
                # bank and needs its own accumulation-group markers
                for mj in range(MJ):
                    nc.tensor.matmul(
                        psum[:, ts(mj, mm_free)],
                        lhsT_of(xt_tiles, rc, ko),
                        w_tiles[ko][:, ts(mj, mm_free)],
                        start=(n_done[rc] == 0),
                        stop=(n_done[rc] == KO - 1),
                    )
                n_done[rc] += 1

            junk_state = {}

            def emit_junk(n):
                for _ in range(n):
                    nc.tensor.matmul(
                        junk_state["ps"][:, ds(0, mm_free)],
                        junk_state["sb"][:, ds(0, P)],
                        junk_state["sb"][:],
                        start=True, stop=True)

            def emit_phase0(w_tiles, o_r, half, rope, xt_tiles, cos_sb, sin_sb,
                            gate_cb=None):
                # Ramped shared pass over the streaming W tiles: rc's join
                # ko-major as their X tiles arrive, skipped ko's caught up
                # afterwards from SBUF-resident W. Junk matmuls plug the
                # W-arrival gaps of the thin early stages to keep HAM warm.
                n_done = [0] * RC
                ps = {r: pspool.tile([P, m_half], F32, tag="ps",
                                     name=f"ps_p0_{r}") for r in JOIN}
                for ko in range(KO):
                    emit_junk(SPRINKLE.get(ko, 0))
                    for rc, j in JOIN.items():
                        if j <= ko:
                            emit_rc_ko(ps[rc], w_tiles, xt_tiles, rc, ko,
                                       n_done)
                o0 = finish_rc(ps[0], o_r, half, 0, rope, cos_sb, sin_sb)
                if gate_cb is not None:
                    gate_cb(o0)
                for rc in sorted(JOIN)[1:]:
                    for ko in range(JOIN[rc]):
                        emit_rc_ko(ps[rc], w_tiles, xt_tiles, rc, ko, n_done)
                    finish_rc(ps[rc], o_r, half, rc, rope, cos_sb, sin_sb)
                for rc in range(len(JOIN), RC):
                    psum = pspool.tile([P, m_half], F32, tag="ps")
                    for ko in range(KO):
                        emit_rc_ko(psum, w_tiles, xt_tiles, rc, ko, n_done)
                    finish_rc(psum, o_r, half, rc, rope, cos_sb, sin_sb)

            def emit_phase(w_tiles, o_r, half, rope, xt_tiles, cos_sb, sin_sb,
                           tail=False):
                n_done = [0] * RC
                for rc in range(RC):
                    psum = pspool.tile([P, m_half], F32, tag="ps")
                    for ko in range(KO):
                        emit_rc_ko(psum, w_tiles, xt_tiles, rc, ko, n_done)
                    if tail and rc == RC - 1:
                        finish_rc_split(psum, o_r, half, rc)
                    else:
                        finish_rc(psum, o_r, half, rc, rope, cos_sb, sin_sb)

            def finish_rc(psum, o_r, half, rc, rope, cos_sb, sin_sb):
                    mc = m_half
                    nh = mc // hd
                    ps = psum[:, ds(0, mc)]
                    o_sb = opool.tile([P, mc], F32, tag="o")
                    if rope:
                        # o = x*cos + swap_pairs(x)*ssin; ssin sign-baked,
                        # the swap is a reversed-stride AP on the pair dim.
                        ps_hd = ps.rearrange("p (h d) -> p h d", d=hd)
                        ps_pr = ps.rearrange(
                            "p (h j two) -> p h j two", h=nh, two=2
                        )
                        cos_b = cos_sb[:, rc, None, :].to_broadcast([P, nh, hd])
                        sin_b = sin_sb[:, rc].rearrange(
                            "p (j two) -> p j two", two=2
                        )[:, None, :, :].to_broadcast([P, nh, J, 2])

                        t_sb = tpool.tile([P, mc], F32, tag="t")
                        t_pr = t_sb[:].rearrange(
                            "p (h j two) -> p h j two", h=nh, two=2
                        )
                        o_hd = o_sb[:].rearrange("p (h d) -> p h d", d=hd)

                        nc.vector.tensor_tensor(
                            t_pr[:], ps_pr[:, :, :, ::-1], sin_b,
                            mybir.AluOpType.mult,
                        )
                        nc.vector.tensor_tensor(
                            o_hd, ps_hd, cos_b, mybir.AluOpType.mult,
                        )
                        nc.vector.tensor_tensor(
                            o_sb[:], o_sb[:], t_sb[:], mybir.AluOpType.add,
                        )
                    else:
                        # DVE copy, not ACT: keeping the program free of
                        # ACTIVATE skips the ACT-preamble table load that
                        # would delay the first W DMA issue by ~1.3us
                        nc.vector.tensor_scalar_add(o_sb[:], ps, 0.0)

                    # stores share the ACT HWDGE ring with the (small,
                    # interleaved) weight prefetches; activations + freqs
                    # own the SP ring so neither queue head-of-line blocks
                    nc.scalar.dma_start(
                        o_r[:, rc, ds(half * m_half, mc)], o_sb[:])
                    return o_sb

            def finish_rc_split(psum, o_r, half, rc):
                # last drain of the kernel: copy per PSUM bank and store on
                # separate rings so the tail after the last matmul is short
                o_sb = opool.tile([P, m_half], F32, tag="o")
                nc.vector.tensor_scalar_add(
                    o_sb[:, ds(0, mm_free)], psum[:, ds(0, mm_free)], 0.0)
                nc.sync.dma_start(
                    o_r[:, rc, ds(half * m_half, mm_free)],
                    o_sb[:, ds(0, mm_free)])
                nc.vector.tensor_scalar_add(
                    o_sb[:, ds(mm_free, mm_free)], psum[:, ds(mm_free, mm_free)],
                    0.0)
                nc.scalar.dma_start(
                    o_r[:, rc, ds(half * m_half + mm_free, mm_free)],
                    o_sb[:, ds(mm_free, mm_free)])

            def body():
                # HAM warmup: dependency-free matmuls on a memset tile run
                # during the engine-init prologue so real matmuls start at
                # 8/8 (2.4GHz) instead of 4/8. (The tile allocator requires
                # a writer for every read tile, so the memset must stay.)
                junk_sb = jpool.tile([P, mm_free], BF16, tag="junk")
                nc.vector.memset(junk_sb[:], 0)
                junk_ps = pspool.tile([P, m_half], F32, tag="ps",
                                      name="junk_ps")
                junk_state["sb"] = junk_sb
                junk_state["ps"] = junk_ps
                emit_junk(JUNK_PRE)

                # V first (no RoPE -> no cos/sin dependency at cold start),
                # V also last (copy-only drain -> short tail)
                phases = []
                for w_dram, o_dram, half, rope in (
                    (wv, v_out, 0, False),
                    (wq, q_out, 0, True),
                    (wq, q_out, 1, True),
                    (wk, k_out, 0, True),
                    (wk, k_out, 1, True),
                    (wv, v_out, 1, False),
                ):
                    w_r = w_dram[:].rearrange("(ko p) m -> p ko m", p=P)
                    o_r = o_dram[:].rearrange("(rc p) m -> p rc m", p=P)
                    phases.append((w_r, o_r, half, rope))

                # x0/x1 in 2 ko-chunks each (first matmul waits on 256KB of
                # X, not 512KB); all X DMAs are contiguous full-rate
                # transfers thanks to the host pre-tiling (x on the SP
                # HWDGE ring, W on ACT's).
                x0c = []
                for c in range(2):
                    x_sb = xpool.tile([P, KO // 2, P], BF16, tag="x0c", bufs=2)
                    nc.sync.dma_start(x_sb[:], xt_r[:, 0, ts(c, KO // 2)])
                    x0c.append(x_sb)
                w_first = load_w_tiles(phases[0][0], phases[0][2])
                x1c = []
                for c in range(2):
                    x_sb = xpool.tile([P, KO // 2, P], BF16, tag="x1c", bufs=2)
                    nc.sync.dma_start(x_sb[:], xt_r[:, 1, ts(c, KO // 2)])
                    x1c.append(x_sb)

                # x tiles >= X_GATED (and cos/sin, needed only from phase 1)
                # are DMA'd only after phase 0's first drain: a dummy DVE
                # write into each tile (data-dependent on that drain) gives
                # the DMA trigger a real WAW dependency, so the early HBM
                # window belongs to the phase-0 W pass + ramp-critical x.
                xt_tiles = [x0c, x1c]
                gated = min(X_GATED, RC) if RC >= len(JOIN) else RC
                for rc in range(2, RC):
                    x_sb = xpool.tile([P, KO, P], BF16, tag="x", bufs=RC - 2)
                    if rc < gated:
                        nc.sync.dma_start(x_sb[:], xt_r[:, rc])
                    xt_tiles.append(x_sb)
                cos_sb = cpool.tile([P, RC, hd], F32, tag="cos")
                sin_sb = cpool.tile([P, RC, hd], F32, tag="sin")
                if gated >= RC:
                    nc.sync.dma_start(cos_sb[:], cos_r)
                    nc.sync.dma_start(sin_sb[:], sin_r)

                def gate_and_load(o_sb_gate):
                    gsrc = o_sb_gate[:, ds(0, 1)]
                    for rc in range(gated, RC):
                        nc.vector.tensor_scalar_add(
                            xt_tiles[rc][:, 0, ds(0, 1)], gsrc, 0.0)
                        nc.sync.dma_start(xt_tiles[rc][:], xt_r[:, rc])
                    nc.vector.tensor_scalar_add(
                        cos_sb[:, 0, ds(0, 1)], gsrc, 0.0)
                    nc.sync.dma_start(cos_sb[:], cos_r)
                    nc.vector.tensor_scalar_add(
                        sin_sb[:, 0, ds(0, 1)], gsrc, 0.0)
                    nc.sync.dma_start(sin_sb[:], sin_r)

                for i, (w_r, o_r, half, rope) in enumerate(phases):
                    w_tiles = w_first if i == 0 else load_w_tiles(w_r, half)
                    if i == 0 and RC >= len(JOIN):
                        emit_phase0(w_tiles, o_r, half, rope, xt_tiles,
                                    cos_sb, sin_sb, gate_cb=gate_and_load)
                    else:
                        emit_phase(w_tiles, o_r, half, rope, xt_tiles,
                                   cos_sb, sin_sb, tail=(i == len(phases) - 1))

            if loop_n == 1:
                body()
            elif unroll:
                for _ in range(loop_n):
                    body()
            else:
                with tc.For_i(0, loop_n, 1):
                    body()

    nc.compile()
    return nc


_NC_CACHE = {}


def _get_nc():
    if "nc" not in _NC_CACHE:
        _NC_CACHE["nc"] = build_nc()
    return _NC_CACHE["nc"]


def prepare_in_maps(X, freqs_cos, freqs_sin, Wq, Wk, Wv):
    X = np.asarray(X, dtype=np.float32)
    freqs_cos = np.asarray(freqs_cos, dtype=np.float32)
    freqs_sin = np.asarray(freqs_sin, dtype=np.float32)

    Xf = X.reshape(B * S, DIM)
    Xb = Xf.astype(ml_dtypes.bfloat16)
    wq_b = np.asarray(Wq, dtype=np.float32).astype(ml_dtypes.bfloat16)
    wk_b = np.asarray(Wk, dtype=np.float32).astype(ml_dtypes.bfloat16)
    wv_b = np.asarray(Wv, dtype=np.float32).astype(ml_dtypes.bfloat16)

    # Rotation sign baked into sin: out[2i] = x[2i]c - x[2i+1]s,
    # out[2i+1] = x[2i+1]c + x[2i]s.
    ssin_full = freqs_sin.copy()
    ssin_full[:, 0::2] *= -1.0

    RCn, KOn = R // P, DIM // P
    in_maps = []
    for c in range(N_CORES):
        rows = slice(c * R, (c + 1) * R)
        s0 = (c % 2) * R  # sequence offset of this shard (R == S // 2)
        # Pre-tile X^T into (rc, p, ko, r) so each device tile DMA is a
        # contiguous [128 x 4KB] transfer: xt[rc*P+p, ko*P+r] = A[rc*P+r,
        # ko*P+p] with A = X-shard [rows, K].
        A = Xb[rows].reshape(RCn, P, KOn, P)
        xt_tiled = np.ascontiguousarray(
            A.transpose(0, 3, 2, 1).reshape(R, DIM))
        in_maps.append({
            "xt": xt_tiled,
            "wq": wq_b,
            "wk": wk_b,
            "wv": wv_b,
            "cosf": np.ascontiguousarray(freqs_cos[s0:s0 + R]),
            "ssin": np.ascontiguousarray(ssin_full[s0:s0 + R]),
        })
    return in_maps


def assemble_outputs(results):
    Xq = np.empty((B * S, H, HD), dtype=np.float32)
    Xk = np.empty((B * S, H, HD), dtype=np.float32)
    Xv = np.empty((B * S, H, HD), dtype=np.float32)
    for c in range(N_CORES):
        rows = slice(c * R, (c + 1) * R)
        Xq[rows] = results[c]["q"].reshape(R, H, HD)
        Xk[rows] = results[c]["k"].reshape(R, H, HD)
        Xv[rows] = results[c]["v"].reshape(R, H, HD)

    return (
        Xq.reshape(B, S, H, HD),
        Xk.reshape(B, S, H, HD),
        Xv.reshape(B, S, H, HD),
    )


def kernel(X, freqs_cos, freqs_sin, attention_mask, Wq, Wk, Wv):
    in_maps = prepare_in_maps(X, freqs_cos, freqs_sin, Wq, Wk, Wv)
    nc = _get_nc()
    res = run_bass_kernel_spmd(nc, in_maps, list(range(N_CORES)))
    return assemble_outputs(res.results)
